# revision 21
# baseline (speedup 1.0000x reference)
"""Multi-head attention (B=2, S=2048, D=1024, H=16) on 8 TRN2 NeuronCores.

Sharding: DP=2 over batch x TP=4 over heads (4 heads/core).
Per core: QKV projections for its 256 output dims, attention for its 4
heads on its batch, row-parallel output projection producing a partial
[2048, 1024] bf16; host sums the 4 partials per batch and adds bo.

v3 layout strategy (per core):
  - everything bf16 except PSUM accumulation and the reciprocal path
  - two-lane beat scheduler for the attention phase: lane0 = (ic, hp=0)
    pairs, lane1 = (ic, hp=1) pairs (staggered). Each beat, each lane
    emits one j-tile (scores pair + exp) and consumes one j-tile
    (attnV accumulate) LAG beats behind, so every PE->ACT->PE handoff
    has a full beat of slack and the ACT engine stays saturated.
  - exp split: most j-tiles on ACT (exp, scale=1/8 folded); 3 j-tiles per
    pair on DVE via a two-term Schraudolph: two tensor_scalar ops
    fp32->int16 (RNE) whose bit patterns are bf16 encodings of
    ~2^t + 2^(t-0.5), summed with one GpSimd bf16 add (~0.5% rms,
    denominators consistent). Their longer latency hides under LAG.
  - v-proj and the q-proj remainder are fed into early-beat PE slack;
    out-proj groups of each finished ic trickle in at 1 per 2 beats.
  - all DMAs on the sync queue; GpSimd runs only the Schraudolph adds
  - attnV via vh_aug [128, 65] (ones column -> denominator row 64)
  - normalization: denominator rows copied to bf16, broadcast to 128
    rows by two K=1 col-tiled matmuls into a borrowed scores-pool slot,
    reciprocal_approx_fast on the broadcast, two muls into `stacked`
"""
import numpy as np

B, S, D = 2, 2048, 1024
HEADS, DK = 16, 64
NCORES, DP, TP = 8, 2, 4
OPC = D // TP          # 256 output dims per core
HPC = HEADS // TP      # 4 heads per core
NDC = D // 128         # 8 contraction chunks
NST = S // 128         # 16 s-tiles (j tiles)
NIC = S // 512         # 4 i-chunks

# two-term Schraudolph constants (see sim_err.py calibration)
SCH_A = float(0.125 * 1.4426950408889634 * 128)
SCH_B1 = 16149.67
SCH_B2 = 16086.17
DVE_JS = (2, 5, 8)     # j-tiles whose exp runs on DVE (early: consumed late)
LAG = 3                # min emitted-ahead before attnV consume
DELAY1 = 4             # lane1 start stagger (pair-serial consumption)

_cache = {}


def _build():
    import concourse.mybir as mybir
    import concourse.tile as tile
    from concourse import bacc

    F32 = mybir.dt.float32
    BF16 = mybir.dt.bfloat16
    I16 = mybir.dt.int16
    Exp = mybir.ActivationFunctionType.Exp
    Mult = mybir.AluOpType.mult
    Add = mybir.AluOpType.add

    nc = bacc.Bacc("TRN2", target_bir_lowering=False, debug=False)

    xq_d = nc.dram_tensor("xqt", [D, S], BF16, kind="ExternalInput")
    xk_d = nc.dram_tensor("xkt", [D, S], BF16, kind="ExternalInput")
    xv_d = nc.dram_tensor("xvt", [D, S], BF16, kind="ExternalInput")
    wq_d = nc.dram_tensor("wqt", [D, OPC], BF16, kind="ExternalInput")
    wk_d = nc.dram_tensor("wkt", [D, OPC], BF16, kind="ExternalInput")
    wv_d = nc.dram_tensor("wvt", [D, OPC], BF16, kind="ExternalInput")
    bq_d = nc.dram_tensor("bq", [2, 128, 1], F32, kind="ExternalInput")
    bk_d = nc.dram_tensor("bk", [2, 128, 1], F32, kind="ExternalInput")
    bv_d = nc.dram_tensor("bv", [128, OPC], F32, kind="ExternalInput")
    wo_d = nc.dram_tensor("wot", [2, 128, D], BF16, kind="ExternalInput")
    out_d = nc.dram_tensor("out", [S, D], BF16, kind="ExternalOutput")

    with tile.TileContext(nc) as tc:
        from contextlib import ExitStack
        es = ExitStack()
        with es:
            wp = es.enter_context(tc.tile_pool(name="wp", bufs=1))
            acts = es.enter_context(tc.tile_pool(name="acts", bufs=1))
            xp = es.enter_context(tc.tile_pool(name="xin", bufs=1))
            ep = es.enter_context(tc.tile_pool(name="ep", bufs=1))
            sps = es.enter_context(tc.tile_pool(name="sps", bufs=2, space="PSUM"))
            mbp = es.enter_context(tc.tile_pool(name="mbp", bufs=2, space="PSUM"))
            pps = es.enter_context(tc.tile_pool(name="pps", bufs=1, space="PSUM"))

            # constants
            ones164 = wp.tile([1, 64], BF16, name="ones164")
            nc.vector.memset(ones164[:], 1.0)

            # persistent activations (qh/kh bf16: f32r streams at ~2cyc/col)
            qh_st = [acts.tile([128, S], BF16, name=f"qh{h}") for h in range(2)]
            kh_st = [acts.tile([128, S], BF16, name=f"kh{h}") for h in range(2)]
            vh_all = acts.tile([128, NST * HPC * 65], BF16, name="vh_all")
            ones_cols = vh_all[:].rearrange("p (g c) -> p g c", c=65)[:, :, 64:65]
            nc.vector.memset(ones_cols, 1.0)
            stacked = [acts.tile([128, S], BF16, name=f"st{h}") for h in range(2)]

            def vh_ap(h, j):
                base = (j * HPC + h) * 65
                return vh_all[:, base:base + 65]

            def dma(dst, src):
                nc.sync.dma_start(dst, src)

            # ---------- DMAs (all up front, sync queue, batched) ----
            def load_w(wd):
                # one DMA: [1024, 256] dram -> [128, 8x256] with dc blocks
                wt = wp.tile([128, NDC * OPC], BF16, name=f"w_{wd.name}")
                dst = wt[:].rearrange("p (dcb c) -> p dcb c", c=OPC)
                srcr = wd.ap().rearrange("(dcb p) c -> p dcb c", p=128)
                dma(dst, srcr)
                return wt

            def w_ap(wt, dc, lo=0, hi=OPC):
                return wt[:, dc * OPC + lo:dc * OPC + hi]

            def load_x_rows(xd, cols=(0, S), tag="xt", w=2048):
                xt = [xp.tile([128, w], BF16, name=tag, tag=tag, bufs=8)
                      for _ in range(NDC)]
                for dc in range(NDC):
                    dma(xt[dc][:, 0:cols[1] - cols[0]],
                        xd.ap()[dc * 128:(dc + 1) * 128, cols[0]:cols[1]])
                return xt

            wk_t = load_w(wk_d)
            xk = load_x_rows(xk_d, tag="xk")
            bk_t = [wp.tile([128, 1], F32, name=f"bk{h}") for h in range(2)]
            for h in range(2):
                dma(bk_t[h][:], bk_d.ap()[h])
            wq_t = load_w(wq_d)
            xq0 = load_x_rows(xq_d, cols=(0, 512), tag="xq0", w=512)
            bq_t = [wp.tile([128, 1], F32, name=f"bq{h}") for h in range(2)]
            for h in range(2):
                dma(bq_t[h][:], bq_d.ap()[h])
            wv_t = load_w(wv_d)
            bv2 = wp.tile([128, OPC], F32, name="bv2")
            dma(bv2[:], bv_d.ap())
            xv = load_x_rows(xv_d, tag="xv")
            xqr = load_x_rows(xq_d, cols=(512, 2048), tag="xqr", w=1536)
            wo_t = [wp.tile([128, D], BF16, name=f"wo{h}") for h in range(2)]
            for h in range(2):
                dma(wo_t[h][:], wo_d.ap()[h])

            # ---------- compute helpers ----------
            def et_tile():
                return ep.tile([128, 1024], BF16, name="et", tag="et", bufs=18)

            def s12_tile():
                return ep.tile([128, 1024], I16, name="s12", tag="s12", bufs=6)

            def qk_proj_group(xs, wt, bias, dest, sc):
                # xs: callable dc -> [128,512] AP of x columns for this sc
                for hp in range(2):
                    p = pps.tile([128, 512], F32, name="pp", tag="pp")
                    for dc in range(NDC):
                        nc.tensor.matmul(
                            p[:], w_ap(wt, dc, hp * 128, (hp + 1) * 128),
                            xs(dc),
                            start=(dc == 0), stop=(dc == NDC - 1))
                    nc.vector.tensor_scalar_add(
                        dest[hp][:, sc * 512:(sc + 1) * 512], p[:],
                        bias[hp][:])

            def k_proj_group(sc):
                qk_proj_group(
                    lambda dc: xk[dc][:, sc * 512:(sc + 1) * 512],
                    wk_t, bk_t, kh_st, sc)

            def q_proj_group(sc):
                if sc == 0:
                    xs = lambda dc: xq0[dc][:]
                else:
                    xs = lambda dc: xqr[dc][:, (sc - 1) * 512:sc * 512]
                qk_proj_group(xs, wq_t, bq_t, qh_st, sc)

            def v_proj_group(st):
                pv = pps.tile([128, OPC], F32, name="pp", tag="pp")
                for dc in range(NDC):
                    nc.tensor.matmul(
                        pv[:], xv[dc][:, st * 128:(st + 1) * 128],
                        w_ap(wv_t, dc), start=(dc == 0), stop=(dc == NDC - 1))
                dst = vh_all[:, st * HPC * 65:(st + 1) * HPC * 65]
                dst = dst.rearrange("p (h c) -> p h c", h=HPC)[:, :, 0:64]
                nc.vector.tensor_add(
                    dst, pv[:].rearrange("p (h c) -> p h c", h=HPC),
                    bv2[:].rearrange("p (h c) -> p h c", h=HPC))

            def scores_pair(hp, ic, j):
                sp = sps.tile([128, 1024], F32, name="sp", tag="sp")
                nc.tensor.matmul(
                    sp[:, 0:512], kh_st[hp][0:64, j * 128:(j + 1) * 128],
                    qh_st[hp][0:64, ic * 512:(ic + 1) * 512],
                    start=True, stop=True, tile_position=(0, 0))
                nc.tensor.matmul(
                    sp[:, 512:1024], kh_st[hp][64:128, j * 128:(j + 1) * 128],
                    qh_st[hp][64:128, ic * 512:(ic + 1) * 512],
                    start=True, stop=True, tile_position=(64, 0))
                return sp

            def exp_tile(sp, j):
                et = et_tile()
                if j in DVE_JS:
                    s1 = s12_tile()
                    s2 = s12_tile()
                    nc.vector.tensor_scalar(s1[:], sp[:], SCH_A, SCH_B1,
                                            Mult, Add)
                    nc.vector.tensor_scalar(s2[:], sp[:], SCH_A, SCH_B2,
                                            Mult, Add)
                    nc.gpsimd.tensor_tensor(
                        et[:], s1[:].bitcast(BF16), s2[:].bitcast(BF16), Add)
                else:
                    nc.scalar.activation(et[:], sp[:], Exp, scale=0.125)
                return et

            def attn_v(av, hp, j, et):
                for h2 in range(2):
                    nc.tensor.matmul(
                        av[h2][0:DK + 1, :], vh_ap(hp * 2 + h2, j),
                        et[:, h2 * 512:(h2 + 1) * 512],
                        start=(j == 0), stop=(j == NST - 1),
                        skip_group_check=True)

            def norm(av, hp, ic):
                den = [ep.tile([1, 512], BF16, name="den", tag="den", bufs=4)
                       for _ in range(2)]
                for h2 in range(2):
                    nc.vector.tensor_copy(den[h2][:], av[h2][DK:DK + 1, :])
                r2 = mbp.tile([128, 512], F32, name="r2", tag="po", bufs=1)
                nc.tensor.matmul(r2[0:64, :], ones164[:], den[0][:],
                                 start=True, stop=True, tile_position=(0, 0))
                nc.tensor.matmul(r2[64:128, :], ones164[:], den[1][:],
                                 start=True, stop=True, tile_position=(0, 64))
                r2s = ep.tile([128, 512], F32, name="r2s", tag="r2s", bufs=2)
                nc.vector.reciprocal_approx_fast(r2s[:], r2[:])
                for h2 in range(2):
                    nc.vector.tensor_mul(
                        stacked[hp][h2 * 64:(h2 + 1) * 64,
                                    ic * 512:(ic + 1) * 512],
                        av[h2][0:DK, :], r2s[h2 * 64:(h2 + 1) * 64, :])

            def po_group(ic, it, mc):
                po = mbp.tile([128, 512], F32, name="po", tag="po", bufs=1)
                for hp in range(2):
                    nc.tensor.matmul(
                        po[:], stacked[hp][:, it * 128:(it + 1) * 128],
                        wo_t[hp][:, mc * 512:(mc + 1) * 512],
                        start=(hp == 0), stop=(hp == 1))
                ot = ep.tile([128, 512], BF16, name="ot", tag="ot", bufs=4)
                nc.vector.tensor_copy(ot[:], po[:])
                dma(out_d.ap()[it * 128:(it + 1) * 128,
                               mc * 512:(mc + 1) * 512], ot[:])

            # ---------- prologue PE work ----------
            for sc in range(4):
                k_proj_group(sc)
            q_proj_group(0)

            # side work queue: v-proj groups early (needed from beat ~8),
            # with the q col-group 1 interleaved; col-groups 2,3 after
            side = [None, None, ("v", 0), ("v", 1), ("q", 1), ("v", 2),
                    ("v", 3)]
            side += [("v", st) for st in range(4, NST)]
            side += [("q", 2), ("q", 3)]

            # ---------- two-lane beat machine ----------
            # lanes emit continuously (1 j-tile scores+exp per beat each);
            # consumption is pair-serial from a fifo at up to 2 j/beat so
            # only ONE pair's av accumulators are live (2 PSUM banks)
            class Lane:
                def __init__(self, hp, delay):
                    self.hp = hp
                    self.delay = delay
                    self.ics = list(range(NIC))
                    self.st = None

                def active(self):
                    return self.st is not None or bool(self.ics)

            lanes = [Lane(0, 0), Lane(1, DELAY1)]
            pair_fifo = []
            norm_done = {ic: 0 for ic in range(NIC)}
            po_queue = []
            beat = 0
            while (any(l.active() for l in lanes) or side or po_queue
                   or pair_fifo):
                for lane in lanes:
                    if lane.delay > 0:
                        lane.delay -= 1
                        continue
                    if lane.st is None:
                        if not lane.ics:
                            continue
                        ic = lane.ics.pop(0)
                        lane.st = {"ic": ic, "hp": lane.hp, "e": 0, "c": 0,
                                   "ets": {}, "av": None}
                        pair_fifo.append(lane.st)
                    st = lane.st
                    j = st["e"]
                    st["ets"][j] = exp_tile(
                        scores_pair(st["hp"], st["ic"], j), j)
                    st["e"] += 1
                    if st["e"] == NST:
                        lane.st = None
                budget = 3
                while budget > 0 and pair_fifo:
                    st = pair_fifo[0]
                    if st["c"] >= NST:
                        pair_fifo.pop(0)
                        continue
                    if not (st["c"] + LAG <= st["e"] or st["e"] == NST):
                        break
                    if st["av"] is None:
                        st["av"] = [mbp.tile([DK + 1, 512], F32, name="av",
                                             tag="av") for _ in range(2)]
                    j = st["c"]
                    attn_v(st["av"], st["hp"], j, st["ets"].pop(j))
                    st["c"] += 1
                    budget -= 1
                    if st["c"] == NST:
                        norm(st["av"], st["hp"], st["ic"])
                        norm_done[st["ic"]] += 1
                        if norm_done[st["ic"]] == 2:
                            po_queue.extend(
                                (st["ic"], it, mc)
                                for it in range(st["ic"] * 4, st["ic"] * 4 + 4)
                                for mc in range(2))
                        pair_fifo.pop(0)
                if side:
                    ent = side.pop(0)
                    if ent is not None:
                        kind, idx = ent
                        if kind == "v":
                            v_proj_group(idx)
                        else:
                            q_proj_group(idx)
                lanes_active = any(l.active() for l in lanes)
                boundary = (pair_fifo and
                            (pair_fifo[0]["c"] >= NST - 3
                             or pair_fifo[0]["c"] <= 1))
                if po_queue and ((beat % 2 == 0 and not boundary)
                                 or not lanes_active):
                    po_group(*po_queue.pop(0))
                beat += 1
            while po_queue:
                po_group(*po_queue.pop(0))

    nc.compile()
    return nc


def _prep_inputs(q, k, v, Wq, bq, Wk, bk, Wv, bv, Wo, bo):
    import ml_dtypes
    f = np.float32
    bf = ml_dtypes.bfloat16
    xT = {}
    for g in range(DP):
        xT[("q", g)] = np.ascontiguousarray(np.asarray(q[g], f).T.astype(bf))
        xT[("k", g)] = np.ascontiguousarray(np.asarray(k[g], f).T.astype(bf))
        xT[("v", g)] = np.ascontiguousarray(np.asarray(v[g], f).T.astype(bf))
    Wq, Wk, Wv, Wo = (np.asarray(a, f) for a in (Wq, Wk, Wv, Wo))
    bq, bk, bv = (np.asarray(a, f) for a in (bq, bk, bv))
    in_maps = []
    for c in range(NCORES):
        g, r = divmod(c, TP)
        sl = slice(r * OPC, (r + 1) * OPC)
        in_maps.append({
            "xqt": xT[("q", g)], "xkt": xT[("k", g)], "xvt": xT[("v", g)],
            "wqt": np.ascontiguousarray(Wq[sl].T.astype(bf)),
            "wkt": np.ascontiguousarray(Wk[sl].T.astype(bf)),
            "wvt": np.ascontiguousarray(Wv[sl].T.astype(bf)),
            "bq": bq[sl].reshape(2, 128, 1),
            "bk": bk[sl].reshape(2, 128, 1),
            "bv": np.ascontiguousarray(np.broadcast_to(bv[sl], (128, OPC))),
            "wot": np.ascontiguousarray(Wo[:, sl].T).reshape(2, 128, D).astype(bf),
        })
    return in_maps


def kernel(q, k, v, Wq, bq, Wk, bk, Wv, bv, Wo, bo, _trace=False):
    from concourse.bass_utils import run_bass_kernel_spmd

    if "nc" not in _cache:
        _cache["nc"] = _build()
    nc = _cache["nc"]
    in_maps = _prep_inputs(q, k, v, Wq, bq, Wk, bk, Wv, bv, Wo, bo)
    res = run_bass_kernel_spmd(nc, in_maps, list(range(NCORES)), trace=_trace)
    _cache["last_exec_time_ns"] = res.exec_time_ns
    _cache["last_res"] = res
    parts = [res.results[c]["out"] for c in range(NCORES)]
    bo = np.asarray(bo, np.float32)
    out = np.empty((B, S, D), np.float32)
    for g in range(DP):
        acc = parts[g * TP].astype(np.float32)
        for r in range(1, TP):
            acc = acc + parts[g * TP + r].astype(np.float32)
        out[g] = acc + bo
    return out


# revision 22
# speedup vs baseline: 1.0053x; 1.0053x over previous
"""Multi-head attention (B=2, S=2048, D=1024, H=16) on 8 TRN2 NeuronCores.

Sharding: DP=2 over batch x TP=4 over heads (4 heads/core).
Per core: QKV projections for its 256 output dims, attention for its 4
heads on its batch, row-parallel output projection producing a partial
[2048, 1024] bf16; host sums the 4 partials per batch and adds bo.

v3 layout strategy (per core):
  - everything bf16 except PSUM accumulation and the reciprocal path
  - two-lane beat scheduler for the attention phase: lane0 = (ic, hp=0)
    pairs, lane1 = (ic, hp=1) pairs (staggered). Each beat, each lane
    emits one j-tile (scores pair + exp) and consumes one j-tile
    (attnV accumulate) LAG beats behind, so every PE->ACT->PE handoff
    has a full beat of slack and the ACT engine stays saturated.
  - exp split: most j-tiles on ACT (exp, scale=1/8 folded); 3 j-tiles per
    pair on DVE via a two-term Schraudolph: two tensor_scalar ops
    fp32->int16 (RNE) whose bit patterns are bf16 encodings of
    ~2^t + 2^(t-0.5), summed with one GpSimd bf16 add (~0.5% rms,
    denominators consistent). Their longer latency hides under LAG.
  - v-proj and the q-proj remainder are fed into early-beat PE slack;
    out-proj groups of each finished ic trickle in at 1 per 2 beats.
  - all DMAs on the sync queue; GpSimd runs only the Schraudolph adds
  - attnV via vh_aug [128, 65] (ones column -> denominator row 64)
  - normalization: denominator rows copied to bf16, broadcast to 128
    rows by two K=1 col-tiled matmuls into a borrowed scores-pool slot,
    reciprocal_approx_fast on the broadcast, two muls into `stacked`
"""
import numpy as np

B, S, D = 2, 2048, 1024
HEADS, DK = 16, 64
NCORES, DP, TP = 8, 2, 4
OPC = D // TP          # 256 output dims per core
HPC = HEADS // TP      # 4 heads per core
NDC = D // 128         # 8 contraction chunks
NST = S // 128         # 16 s-tiles (j tiles)
NIC = S // 512         # 4 i-chunks

# two-term Schraudolph constants (see sim_err.py calibration)
SCH_A = float(0.125 * 1.4426950408889634 * 128)
SCH_B1 = 16149.67
SCH_B2 = 16086.17
DVE_JS = (7, 10, 13)   # j-tiles whose exp runs on DVE (mid-late: chain hides)
LAG = 3                # min emitted-ahead before attnV consume
DELAY1 = 4             # lane1 start stagger (pair-serial consumption)

_cache = {}


def _build():
    import concourse.mybir as mybir
    import concourse.tile as tile
    from concourse import bacc

    F32 = mybir.dt.float32
    BF16 = mybir.dt.bfloat16
    I16 = mybir.dt.int16
    Exp = mybir.ActivationFunctionType.Exp
    Mult = mybir.AluOpType.mult
    Add = mybir.AluOpType.add

    nc = bacc.Bacc("TRN2", target_bir_lowering=False, debug=False)

    xq_d = nc.dram_tensor("xqt", [D, S], BF16, kind="ExternalInput")
    xk_d = nc.dram_tensor("xkt", [D, S], BF16, kind="ExternalInput")
    xv_d = nc.dram_tensor("xvt", [D, S], BF16, kind="ExternalInput")
    wq_d = nc.dram_tensor("wqt", [D, OPC], BF16, kind="ExternalInput")
    wk_d = nc.dram_tensor("wkt", [D, OPC], BF16, kind="ExternalInput")
    wv_d = nc.dram_tensor("wvt", [D, OPC], BF16, kind="ExternalInput")
    bq_d = nc.dram_tensor("bq", [2, 128, 1], F32, kind="ExternalInput")
    bk_d = nc.dram_tensor("bk", [2, 128, 1], F32, kind="ExternalInput")
    bv_d = nc.dram_tensor("bv", [128, OPC], F32, kind="ExternalInput")
    wo_d = nc.dram_tensor("wot", [2, 128, D], BF16, kind="ExternalInput")
    out_d = nc.dram_tensor("out", [S, D], BF16, kind="ExternalOutput")

    with tile.TileContext(nc) as tc:
        from contextlib import ExitStack
        es = ExitStack()
        with es:
            wp = es.enter_context(tc.tile_pool(name="wp", bufs=1))
            acts = es.enter_context(tc.tile_pool(name="acts", bufs=1))
            xp = es.enter_context(tc.tile_pool(name="xin", bufs=1))
            ep = es.enter_context(tc.tile_pool(name="ep", bufs=1))
            sps = es.enter_context(tc.tile_pool(name="sps", bufs=2, space="PSUM"))
            mbp = es.enter_context(tc.tile_pool(name="mbp", bufs=2, space="PSUM"))
            pps = es.enter_context(tc.tile_pool(name="pps", bufs=1, space="PSUM"))

            # constants
            ones164 = wp.tile([1, 64], BF16, name="ones164")
            nc.vector.memset(ones164[:], 1.0)

            # persistent activations (qh/kh bf16: f32r streams at ~2cyc/col)
            qh_st = [acts.tile([128, S], BF16, name=f"qh{h}") for h in range(2)]
            kh_st = [acts.tile([128, S], BF16, name=f"kh{h}") for h in range(2)]
            vh_all = acts.tile([128, NST * HPC * 65], BF16, name="vh_all")
            ones_cols = vh_all[:].rearrange("p (g c) -> p g c", c=65)[:, :, 64:65]
            nc.vector.memset(ones_cols, 1.0)
            stacked = [acts.tile([128, S], BF16, name=f"st{h}") for h in range(2)]

            def vh_ap(h, j):
                base = (j * HPC + h) * 65
                return vh_all[:, base:base + 65]

            def dma(dst, src):
                nc.sync.dma_start(dst, src)

            # ---------- DMAs (all up front, sync queue, batched) ----
            def load_w(wd):
                # one DMA: [1024, 256] dram -> [128, 8x256] with dc blocks
                wt = wp.tile([128, NDC * OPC], BF16, name=f"w_{wd.name}")
                dst = wt[:].rearrange("p (dcb c) -> p dcb c", c=OPC)
                srcr = wd.ap().rearrange("(dcb p) c -> p dcb c", p=128)
                dma(dst, srcr)
                return wt

            def w_ap(wt, dc, lo=0, hi=OPC):
                return wt[:, dc * OPC + lo:dc * OPC + hi]

            def load_x_rows(xd, cols=(0, S), tag="xt", w=2048):
                xt = [xp.tile([128, w], BF16, name=tag, tag=tag, bufs=8)
                      for _ in range(NDC)]
                for dc in range(NDC):
                    dma(xt[dc][:, 0:cols[1] - cols[0]],
                        xd.ap()[dc * 128:(dc + 1) * 128, cols[0]:cols[1]])
                return xt

            wk_t = load_w(wk_d)
            xk = load_x_rows(xk_d, tag="xk")
            bk_t = [wp.tile([128, 1], F32, name=f"bk{h}") for h in range(2)]
            for h in range(2):
                dma(bk_t[h][:], bk_d.ap()[h])
            wq_t = load_w(wq_d)
            xq0 = load_x_rows(xq_d, cols=(0, 512), tag="xq0", w=512)
            bq_t = [wp.tile([128, 1], F32, name=f"bq{h}") for h in range(2)]
            for h in range(2):
                dma(bq_t[h][:], bq_d.ap()[h])
            wv_t = load_w(wv_d)
            bv2 = wp.tile([128, OPC], F32, name="bv2")
            dma(bv2[:], bv_d.ap())
            xv = load_x_rows(xv_d, tag="xv")
            xqr = load_x_rows(xq_d, cols=(512, 2048), tag="xqr", w=1536)
            wo_t = [wp.tile([128, D], BF16, name=f"wo{h}") for h in range(2)]
            for h in range(2):
                dma(wo_t[h][:], wo_d.ap()[h])

            # ---------- compute helpers ----------
            def et_tile():
                return ep.tile([128, 1024], BF16, name="et", tag="et", bufs=18)

            def s12_tile():
                return ep.tile([128, 1024], I16, name="s12", tag="s12", bufs=6)

            def qk_proj_group(xs, wt, bias, dest, sc):
                # xs: callable dc -> [128,512] AP of x columns for this sc
                for hp in range(2):
                    p = pps.tile([128, 512], F32, name="pp", tag="pp")
                    for dc in range(NDC):
                        nc.tensor.matmul(
                            p[:], w_ap(wt, dc, hp * 128, (hp + 1) * 128),
                            xs(dc),
                            start=(dc == 0), stop=(dc == NDC - 1))
                    nc.vector.tensor_scalar_add(
                        dest[hp][:, sc * 512:(sc + 1) * 512], p[:],
                        bias[hp][:])

            def k_proj_group(sc):
                qk_proj_group(
                    lambda dc: xk[dc][:, sc * 512:(sc + 1) * 512],
                    wk_t, bk_t, kh_st, sc)

            def q_proj_group(sc):
                if sc == 0:
                    xs = lambda dc: xq0[dc][:]
                else:
                    xs = lambda dc: xqr[dc][:, (sc - 1) * 512:sc * 512]
                qk_proj_group(xs, wq_t, bq_t, qh_st, sc)

            def v_proj_group(st):
                pv = pps.tile([128, OPC], F32, name="pp", tag="pp")
                for dc in range(NDC):
                    nc.tensor.matmul(
                        pv[:], xv[dc][:, st * 128:(st + 1) * 128],
                        w_ap(wv_t, dc), start=(dc == 0), stop=(dc == NDC - 1))
                dst = vh_all[:, st * HPC * 65:(st + 1) * HPC * 65]
                dst = dst.rearrange("p (h c) -> p h c", h=HPC)[:, :, 0:64]
                nc.vector.tensor_add(
                    dst, pv[:].rearrange("p (h c) -> p h c", h=HPC),
                    bv2[:].rearrange("p (h c) -> p h c", h=HPC))

            def scores_pair(hp, ic, j):
                sp = sps.tile([128, 1024], F32, name="sp", tag="sp")
                nc.tensor.matmul(
                    sp[:, 0:512], kh_st[hp][0:64, j * 128:(j + 1) * 128],
                    qh_st[hp][0:64, ic * 512:(ic + 1) * 512],
                    start=True, stop=True, tile_position=(0, 0))
                nc.tensor.matmul(
                    sp[:, 512:1024], kh_st[hp][64:128, j * 128:(j + 1) * 128],
                    qh_st[hp][64:128, ic * 512:(ic + 1) * 512],
                    start=True, stop=True, tile_position=(64, 0))
                return sp

            def exp_tile(sp, j):
                et = et_tile()
                if j in DVE_JS:
                    s1 = s12_tile()
                    s2 = s12_tile()
                    nc.vector.tensor_scalar(s1[:], sp[:], SCH_A, SCH_B1,
                                            Mult, Add)
                    nc.vector.tensor_scalar(s2[:], sp[:], SCH_A, SCH_B2,
                                            Mult, Add)
                    nc.gpsimd.tensor_tensor(
                        et[:], s1[:].bitcast(BF16), s2[:].bitcast(BF16), Add)
                else:
                    nc.scalar.activation(et[:], sp[:], Exp, scale=0.125)
                return et

            def attn_v(av, hp, j, et):
                for h2 in range(2):
                    nc.tensor.matmul(
                        av[h2][0:DK + 1, :], vh_ap(hp * 2 + h2, j),
                        et[:, h2 * 512:(h2 + 1) * 512],
                        start=(j == 0), stop=(j == NST - 1),
                        skip_group_check=True)

            def norm(av, hp, ic):
                den = [ep.tile([1, 512], BF16, name="den", tag="den", bufs=4)
                       for _ in range(2)]
                for h2 in range(2):
                    nc.vector.tensor_copy(den[h2][:], av[h2][DK:DK + 1, :])
                r2 = mbp.tile([128, 512], F32, name="r2", tag="po", bufs=1)
                nc.tensor.matmul(r2[0:64, :], ones164[:], den[0][:],
                                 start=True, stop=True, tile_position=(0, 0))
                nc.tensor.matmul(r2[64:128, :], ones164[:], den[1][:],
                                 start=True, stop=True, tile_position=(0, 64))
                r2s = ep.tile([128, 512], F32, name="r2s", tag="r2s", bufs=2)
                nc.vector.reciprocal_approx_fast(r2s[:], r2[:])
                for h2 in range(2):
                    nc.vector.tensor_mul(
                        stacked[hp][h2 * 64:(h2 + 1) * 64,
                                    ic * 512:(ic + 1) * 512],
                        av[h2][0:DK, :], r2s[h2 * 64:(h2 + 1) * 64, :])

            def po_group(ic, it, mc):
                po = mbp.tile([128, 512], F32, name="po", tag="po", bufs=1)
                for hp in range(2):
                    nc.tensor.matmul(
                        po[:], stacked[hp][:, it * 128:(it + 1) * 128],
                        wo_t[hp][:, mc * 512:(mc + 1) * 512],
                        start=(hp == 0), stop=(hp == 1))
                ot = ep.tile([128, 512], BF16, name="ot", tag="ot", bufs=4)
                nc.vector.tensor_copy(ot[:], po[:])
                dma(out_d.ap()[it * 128:(it + 1) * 128,
                               mc * 512:(mc + 1) * 512], ot[:])

            # ---------- prologue PE work ----------
            for sc in range(4):
                k_proj_group(sc)
            q_proj_group(0)

            # side work queue: v-proj groups early (needed from beat ~8),
            # with the q col-group 1 interleaved; col-groups 2,3 after
            side = [None, None, ("v", 0), ("v", 1), ("q", 1), ("v", 2),
                    ("v", 3)]
            side += [("v", st) for st in range(4, NST)]
            side += [("q", 2), ("q", 3)]

            # ---------- two-lane beat machine ----------
            # lanes emit continuously (1 j-tile scores+exp per beat each);
            # consumption is pair-serial from a fifo at up to 2 j/beat so
            # only ONE pair's av accumulators are live (2 PSUM banks)
            class Lane:
                def __init__(self, hp, delay):
                    self.hp = hp
                    self.delay = delay
                    self.ics = list(range(NIC))
                    self.st = None

                def active(self):
                    return self.st is not None or bool(self.ics)

            lanes = [Lane(0, 0), Lane(1, DELAY1)]
            pair_fifo = []
            norm_done = {ic: 0 for ic in range(NIC)}
            po_queue = []
            beat = 0
            while (any(l.active() for l in lanes) or side or po_queue
                   or pair_fifo):
                for lane in lanes:
                    if lane.delay > 0:
                        lane.delay -= 1
                        continue
                    if lane.st is None:
                        if not lane.ics:
                            continue
                        ic = lane.ics.pop(0)
                        lane.st = {"ic": ic, "hp": lane.hp, "e": 0, "c": 0,
                                   "ets": {}, "av": None}
                        pair_fifo.append(lane.st)
                    st = lane.st
                    j = st["e"]
                    st["ets"][j] = exp_tile(
                        scores_pair(st["hp"], st["ic"], j), j)
                    st["e"] += 1
                    if st["e"] == NST:
                        lane.st = None
                budget = 3
                while budget > 0 and pair_fifo:
                    st = pair_fifo[0]
                    if st["c"] >= NST:
                        pair_fifo.pop(0)
                        continue
                    if not (st["c"] + LAG <= st["e"] or st["e"] == NST):
                        break
                    if st["av"] is None:
                        st["av"] = [mbp.tile([DK + 1, 512], F32, name="av",
                                             tag="av") for _ in range(2)]
                    j = st["c"]
                    attn_v(st["av"], st["hp"], j, st["ets"].pop(j))
                    st["c"] += 1
                    budget -= 1
                    if st["c"] == NST:
                        norm(st["av"], st["hp"], st["ic"])
                        norm_done[st["ic"]] += 1
                        if norm_done[st["ic"]] == 2:
                            po_queue.extend(
                                (st["ic"], it, mc)
                                for it in range(st["ic"] * 4, st["ic"] * 4 + 4)
                                for mc in range(2))
                        pair_fifo.pop(0)
                if side:
                    ent = side.pop(0)
                    if ent is not None:
                        kind, idx = ent
                        if kind == "v":
                            v_proj_group(idx)
                        else:
                            q_proj_group(idx)
                lanes_active = any(l.active() for l in lanes)
                boundary = (pair_fifo and
                            (pair_fifo[0]["c"] >= NST - 3
                             or pair_fifo[0]["c"] <= 1))
                if po_queue and ((beat % 2 == 0 and not boundary)
                                 or not lanes_active):
                    po_group(*po_queue.pop(0))
                if not side and lanes_active:
                    # HAM warm-keeper: a ~70ns matmul per beat so the PE
                    # activity monitor never sees an idle window
                    dm = pps.tile([64, 64], F32, name="dummy", tag="pp")
                    nc.tensor.matmul(dm[:], ones164[:], ones164[:],
                                     start=True, stop=True)
                beat += 1
            while po_queue:
                po_group(*po_queue.pop(0))

    nc.compile()
    return nc


def _prep_inputs(q, k, v, Wq, bq, Wk, bk, Wv, bv, Wo, bo):
    import ml_dtypes
    f = np.float32
    bf = ml_dtypes.bfloat16
    xT = {}
    for g in range(DP):
        xT[("q", g)] = np.ascontiguousarray(np.asarray(q[g], f).T.astype(bf))
        xT[("k", g)] = np.ascontiguousarray(np.asarray(k[g], f).T.astype(bf))
        xT[("v", g)] = np.ascontiguousarray(np.asarray(v[g], f).T.astype(bf))
    Wq, Wk, Wv, Wo = (np.asarray(a, f) for a in (Wq, Wk, Wv, Wo))
    bq, bk, bv = (np.asarray(a, f) for a in (bq, bk, bv))
    in_maps = []
    for c in range(NCORES):
        g, r = divmod(c, TP)
        sl = slice(r * OPC, (r + 1) * OPC)
        in_maps.append({
            "xqt": xT[("q", g)], "xkt": xT[("k", g)], "xvt": xT[("v", g)],
            "wqt": np.ascontiguousarray(Wq[sl].T.astype(bf)),
            "wkt": np.ascontiguousarray(Wk[sl].T.astype(bf)),
            "wvt": np.ascontiguousarray(Wv[sl].T.astype(bf)),
            "bq": bq[sl].reshape(2, 128, 1),
            "bk": bk[sl].reshape(2, 128, 1),
            "bv": np.ascontiguousarray(np.broadcast_to(bv[sl], (128, OPC))),
            "wot": np.ascontiguousarray(Wo[:, sl].T).reshape(2, 128, D).astype(bf),
        })
    return in_maps


def kernel(q, k, v, Wq, bq, Wk, bk, Wv, bv, Wo, bo, _trace=False):
    from concourse.bass_utils import run_bass_kernel_spmd

    if "nc" not in _cache:
        _cache["nc"] = _build()
    nc = _cache["nc"]
    in_maps = _prep_inputs(q, k, v, Wq, bq, Wk, bk, Wv, bv, Wo, bo)
    res = run_bass_kernel_spmd(nc, in_maps, list(range(NCORES)), trace=_trace)
    _cache["last_exec_time_ns"] = res.exec_time_ns
    _cache["last_res"] = res
    parts = [res.results[c]["out"] for c in range(NCORES)]
    bo = np.asarray(bo, np.float32)
    out = np.empty((B, S, D), np.float32)
    for g in range(DP):
        acc = parts[g * TP].astype(np.float32)
        for r in range(1, TP):
            acc = acc + parts[g * TP + r].astype(np.float32)
        out[g] = acc + bo
    return out


# revision 23
# speedup vs baseline: 1.0197x; 1.0144x over previous
"""Multi-head attention (B=2, S=2048, D=1024, H=16) on 8 TRN2 NeuronCores.

Sharding: DP=2 over batch x TP=4 over heads (4 heads/core).
Per core: QKV projections for its 256 output dims, attention for its 4
heads on its batch, row-parallel output projection producing a partial
[2048, 1024] bf16; host sums the 4 partials per batch and adds bo.

v3 layout strategy (per core):
  - everything bf16 except PSUM accumulation and the reciprocal path
  - two-lane beat scheduler for the attention phase: lane0 = (ic, hp=0)
    pairs, lane1 = (ic, hp=1) pairs (staggered). Each beat, each lane
    emits one j-tile (scores pair + exp) and consumes one j-tile
    (attnV accumulate) LAG beats behind, so every PE->ACT->PE handoff
    has a full beat of slack and the ACT engine stays saturated.
  - exp split: most j-tiles on ACT (exp, scale=1/8 folded); 3 j-tiles per
    pair on DVE via a two-term Schraudolph: two tensor_scalar ops
    fp32->int16 (RNE) whose bit patterns are bf16 encodings of
    ~2^t + 2^(t-0.5), summed with one GpSimd bf16 add (~0.5% rms,
    denominators consistent). Their longer latency hides under LAG.
  - v-proj and the q-proj remainder are fed into early-beat PE slack;
    out-proj groups of each finished ic trickle in at 1 per 2 beats.
  - all DMAs on the sync queue; GpSimd runs only the Schraudolph adds
  - attnV via vh_aug [128, 65] (ones column -> denominator row 64)
  - normalization: denominator rows copied to bf16, broadcast to 128
    rows by two K=1 col-tiled matmuls into a borrowed scores-pool slot,
    reciprocal_approx_fast on the broadcast, two muls into `stacked`
"""
import numpy as np

B, S, D = 2, 2048, 1024
HEADS, DK = 16, 64
NCORES, DP, TP = 8, 2, 4
OPC = D // TP          # 256 output dims per core
HPC = HEADS // TP      # 4 heads per core
NDC = D // 128         # 8 contraction chunks
NST = S // 128         # 16 s-tiles (j tiles)
NIC = S // 512         # 4 i-chunks

# two-term Schraudolph constants (see sim_err.py calibration)
SCH_A = float(0.125 * 1.4426950408889634 * 128)
SCH_B1 = 16149.67
SCH_B2 = 16086.17
DVE_JS = (7, 10, 13)   # j-tiles whose exp runs on DVE (mid-late: chain hides)
LAG = 5                # min emitted-ahead before attnV consume
DELAY1 = 8             # lane1 start stagger (pair-serial consumption)

_cache = {}


def _build():
    import concourse.mybir as mybir
    import concourse.tile as tile
    from concourse import bacc

    F32 = mybir.dt.float32
    BF16 = mybir.dt.bfloat16
    I16 = mybir.dt.int16
    Exp = mybir.ActivationFunctionType.Exp
    Mult = mybir.AluOpType.mult
    Add = mybir.AluOpType.add

    nc = bacc.Bacc("TRN2", target_bir_lowering=False, debug=False)

    xq_d = nc.dram_tensor("xqt", [D, S], BF16, kind="ExternalInput")
    xk_d = nc.dram_tensor("xkt", [D, S], BF16, kind="ExternalInput")
    xv_d = nc.dram_tensor("xvt", [D, S], BF16, kind="ExternalInput")
    wq_d = nc.dram_tensor("wqt", [D, OPC], BF16, kind="ExternalInput")
    wk_d = nc.dram_tensor("wkt", [D, OPC], BF16, kind="ExternalInput")
    wv_d = nc.dram_tensor("wvt", [D, OPC], BF16, kind="ExternalInput")
    bq_d = nc.dram_tensor("bq", [2, 128, 1], F32, kind="ExternalInput")
    bk_d = nc.dram_tensor("bk", [2, 128, 1], F32, kind="ExternalInput")
    bv_d = nc.dram_tensor("bv", [128, OPC], F32, kind="ExternalInput")
    wo_d = nc.dram_tensor("wot", [2, 128, D], BF16, kind="ExternalInput")
    out_d = nc.dram_tensor("out", [S, D], BF16, kind="ExternalOutput")

    with tile.TileContext(nc) as tc:
        from contextlib import ExitStack
        es = ExitStack()
        with es:
            wp = es.enter_context(tc.tile_pool(name="wp", bufs=1))
            acts = es.enter_context(tc.tile_pool(name="acts", bufs=1))
            xp = es.enter_context(tc.tile_pool(name="xin", bufs=1))
            ep = es.enter_context(tc.tile_pool(name="ep", bufs=1))
            sps = es.enter_context(tc.tile_pool(name="sps", bufs=2, space="PSUM"))
            mbp = es.enter_context(tc.tile_pool(name="mbp", bufs=2, space="PSUM"))
            pps = es.enter_context(tc.tile_pool(name="pps", bufs=1, space="PSUM"))

            # constants
            ones164 = wp.tile([1, 64], BF16, name="ones164")
            nc.vector.memset(ones164[:], 1.0)

            # persistent activations (qh/kh bf16: f32r streams at ~2cyc/col)
            qh_st = [acts.tile([128, S], BF16, name=f"qh{h}") for h in range(2)]
            kh_st = [acts.tile([128, S], BF16, name=f"kh{h}") for h in range(2)]
            vh_all = acts.tile([128, NST * HPC * 65], BF16, name="vh_all")
            ones_cols = vh_all[:].rearrange("p (g c) -> p g c", c=65)[:, :, 64:65]
            nc.vector.memset(ones_cols, 1.0)
            stacked = [acts.tile([128, S], BF16, name=f"st{h}") for h in range(2)]

            def vh_ap(h, j):
                base = (j * HPC + h) * 65
                return vh_all[:, base:base + 65]

            def dma(dst, src):
                nc.sync.dma_start(dst, src)

            # ---------- DMAs (all up front, sync queue, batched) ----
            def load_w(wd):
                # one DMA: [1024, 256] dram -> [128, 8x256] with dc blocks
                wt = wp.tile([128, NDC * OPC], BF16, name=f"w_{wd.name}")
                dst = wt[:].rearrange("p (dcb c) -> p dcb c", c=OPC)
                srcr = wd.ap().rearrange("(dcb p) c -> p dcb c", p=128)
                dma(dst, srcr)
                return wt

            def w_ap(wt, dc, lo=0, hi=OPC):
                return wt[:, dc * OPC + lo:dc * OPC + hi]

            def load_x_rows(xd, cols=(0, S), tag="xt", w=2048):
                xt = [xp.tile([128, w], BF16, name=tag, tag=tag, bufs=8)
                      for _ in range(NDC)]
                for dc in range(NDC):
                    dma(xt[dc][:, 0:cols[1] - cols[0]],
                        xd.ap()[dc * 128:(dc + 1) * 128, cols[0]:cols[1]])
                return xt

            wk_t = load_w(wk_d)
            xk = load_x_rows(xk_d, tag="xk")
            bk_t = [wp.tile([128, 1], F32, name=f"bk{h}") for h in range(2)]
            for h in range(2):
                dma(bk_t[h][:], bk_d.ap()[h])
            wq_t = load_w(wq_d)
            xq0 = load_x_rows(xq_d, cols=(0, 512), tag="xq0", w=512)
            bq_t = [wp.tile([128, 1], F32, name=f"bq{h}") for h in range(2)]
            for h in range(2):
                dma(bq_t[h][:], bq_d.ap()[h])
            wv_t = load_w(wv_d)
            bv2 = wp.tile([128, OPC], F32, name="bv2")
            dma(bv2[:], bv_d.ap())
            xv = load_x_rows(xv_d, tag="xv")
            xqr = load_x_rows(xq_d, cols=(512, 2048), tag="xqr", w=1536)
            wo_t = [wp.tile([128, D], BF16, name=f"wo{h}") for h in range(2)]
            for h in range(2):
                dma(wo_t[h][:], wo_d.ap()[h])

            # ---------- compute helpers ----------
            def et_tile():
                return ep.tile([128, 1024], BF16, name="et", tag="et", bufs=18)

            def s12_tile():
                return ep.tile([128, 1024], I16, name="s12", tag="s12", bufs=6)

            def qk_proj_group(xs, wt, bias, dest, sc):
                # xs: callable dc -> [128,512] AP of x columns for this sc
                for hp in range(2):
                    p = pps.tile([128, 512], F32, name="pp", tag="pp")
                    for dc in range(NDC):
                        nc.tensor.matmul(
                            p[:], w_ap(wt, dc, hp * 128, (hp + 1) * 128),
                            xs(dc),
                            start=(dc == 0), stop=(dc == NDC - 1))
                    nc.vector.tensor_scalar_add(
                        dest[hp][:, sc * 512:(sc + 1) * 512], p[:],
                        bias[hp][:])

            def k_proj_group(sc):
                qk_proj_group(
                    lambda dc: xk[dc][:, sc * 512:(sc + 1) * 512],
                    wk_t, bk_t, kh_st, sc)

            def q_proj_group(sc):
                if sc == 0:
                    xs = lambda dc: xq0[dc][:]
                else:
                    xs = lambda dc: xqr[dc][:, (sc - 1) * 512:sc * 512]
                qk_proj_group(xs, wq_t, bq_t, qh_st, sc)

            def v_proj_group(st):
                pv = pps.tile([128, OPC], F32, name="pp", tag="pp")
                for dc in range(NDC):
                    nc.tensor.matmul(
                        pv[:], xv[dc][:, st * 128:(st + 1) * 128],
                        w_ap(wv_t, dc), start=(dc == 0), stop=(dc == NDC - 1))
                dst = vh_all[:, st * HPC * 65:(st + 1) * HPC * 65]
                dst = dst.rearrange("p (h c) -> p h c", h=HPC)[:, :, 0:64]
                nc.vector.tensor_add(
                    dst, pv[:].rearrange("p (h c) -> p h c", h=HPC),
                    bv2[:].rearrange("p (h c) -> p h c", h=HPC))

            def scores_pair(hp, ic, j):
                sp = sps.tile([128, 1024], F32, name="sp", tag="sp")
                nc.tensor.matmul(
                    sp[:, 0:512], kh_st[hp][0:64, j * 128:(j + 1) * 128],
                    qh_st[hp][0:64, ic * 512:(ic + 1) * 512],
                    start=True, stop=True, tile_position=(0, 0))
                nc.tensor.matmul(
                    sp[:, 512:1024], kh_st[hp][64:128, j * 128:(j + 1) * 128],
                    qh_st[hp][64:128, ic * 512:(ic + 1) * 512],
                    start=True, stop=True, tile_position=(64, 0))
                return sp

            def exp_tile(sp, j):
                et = et_tile()
                if j in DVE_JS:
                    s1 = s12_tile()
                    s2 = s12_tile()
                    nc.vector.tensor_scalar(s1[:], sp[:], SCH_A, SCH_B1,
                                            Mult, Add)
                    nc.vector.tensor_scalar(s2[:], sp[:], SCH_A, SCH_B2,
                                            Mult, Add)
                    nc.gpsimd.tensor_tensor(
                        et[:], s1[:].bitcast(BF16), s2[:].bitcast(BF16), Add)
                else:
                    nc.scalar.activation(et[:], sp[:], Exp, scale=0.125)
                return et

            def attn_v(av, hp, j, et):
                for h2 in range(2):
                    nc.tensor.matmul(
                        av[h2][0:DK + 1, :], vh_ap(hp * 2 + h2, j),
                        et[:, h2 * 512:(h2 + 1) * 512],
                        start=(j == 0), stop=(j == NST - 1),
                        skip_group_check=True)

            def norm(av, hp, ic):
                den = [ep.tile([1, 512], BF16, name="den", tag="den", bufs=4)
                       for _ in range(2)]
                for h2 in range(2):
                    nc.vector.tensor_copy(den[h2][:], av[h2][DK:DK + 1, :])
                r2 = mbp.tile([128, 512], F32, name="r2", tag="po", bufs=1)
                nc.tensor.matmul(r2[0:64, :], ones164[:], den[0][:],
                                 start=True, stop=True, tile_position=(0, 0))
                nc.tensor.matmul(r2[64:128, :], ones164[:], den[1][:],
                                 start=True, stop=True, tile_position=(0, 64))
                r2s = ep.tile([128, 512], F32, name="r2s", tag="r2s", bufs=2)
                nc.vector.reciprocal_approx_fast(r2s[:], r2[:])
                for h2 in range(2):
                    nc.vector.tensor_mul(
                        stacked[hp][h2 * 64:(h2 + 1) * 64,
                                    ic * 512:(ic + 1) * 512],
                        av[h2][0:DK, :], r2s[h2 * 64:(h2 + 1) * 64, :])

            def po_group(ic, it, mc):
                po = mbp.tile([128, 512], F32, name="po", tag="po", bufs=1)
                for hp in range(2):
                    nc.tensor.matmul(
                        po[:], stacked[hp][:, it * 128:(it + 1) * 128],
                        wo_t[hp][:, mc * 512:(mc + 1) * 512],
                        start=(hp == 0), stop=(hp == 1))
                ot = ep.tile([128, 512], BF16, name="ot", tag="ot", bufs=4)
                nc.vector.tensor_copy(ot[:], po[:])
                dma(out_d.ap()[it * 128:(it + 1) * 128,
                               mc * 512:(mc + 1) * 512], ot[:])

            # ---------- prologue PE work ----------
            for sc in range(4):
                k_proj_group(sc)
            q_proj_group(0)

            # side work queue: v-proj groups early (needed from beat ~8),
            # with the q col-group 1 interleaved; col-groups 2,3 after
            side = [None, None, ("v", 0), ("v", 1), ("q", 1), ("v", 2),
                    ("v", 3)]
            side += [("v", st) for st in range(4, NST)]
            side += [("q", 2), ("q", 3)]

            # ---------- two-lane beat machine ----------
            # lanes emit continuously (1 j-tile scores+exp per beat each);
            # consumption is pair-serial from a fifo at up to 2 j/beat so
            # only ONE pair's av accumulators are live (2 PSUM banks)
            class Lane:
                def __init__(self, hp, delay):
                    self.hp = hp
                    self.delay = delay
                    self.ics = list(range(NIC))
                    self.st = None

                def active(self):
                    return self.st is not None or bool(self.ics)

            lanes = [Lane(0, 0), Lane(1, DELAY1)]
            pair_fifo = []
            norm_done = {ic: 0 for ic in range(NIC)}
            po_queue = []
            beat = 0
            while (any(l.active() for l in lanes) or side or po_queue
                   or pair_fifo):
                for lane in lanes:
                    if lane.delay > 0:
                        lane.delay -= 1
                        continue
                    if lane.st is None:
                        if not lane.ics:
                            continue
                        ic = lane.ics.pop(0)
                        lane.st = {"ic": ic, "hp": lane.hp, "e": 0, "c": 0,
                                   "ets": {}, "av": None}
                        pair_fifo.append(lane.st)
                    st = lane.st
                    j = st["e"]
                    st["ets"][j] = exp_tile(
                        scores_pair(st["hp"], st["ic"], j), j)
                    st["e"] += 1
                    if st["e"] == NST:
                        lane.st = None
                budget = 2
                while budget > 0 and pair_fifo:
                    st = pair_fifo[0]
                    if st["c"] >= NST:
                        pair_fifo.pop(0)
                        continue
                    if not (st["c"] + LAG <= st["e"] or st["e"] == NST):
                        break
                    if st["av"] is None:
                        st["av"] = [mbp.tile([DK + 1, 512], F32, name="av",
                                             tag="av") for _ in range(2)]
                    j = st["c"]
                    attn_v(st["av"], st["hp"], j, st["ets"].pop(j))
                    st["c"] += 1
                    budget -= 1
                    if st["c"] == NST:
                        norm(st["av"], st["hp"], st["ic"])
                        norm_done[st["ic"]] += 1
                        if norm_done[st["ic"]] == 2:
                            po_queue.extend(
                                (st["ic"], it, mc)
                                for it in range(st["ic"] * 4, st["ic"] * 4 + 4)
                                for mc in range(2))
                        pair_fifo.pop(0)
                if side:
                    ent = side.pop(0)
                    if ent is not None:
                        kind, idx = ent
                        if kind == "v":
                            v_proj_group(idx)
                        else:
                            q_proj_group(idx)
                lanes_active = any(l.active() for l in lanes)
                boundary = (pair_fifo and
                            (pair_fifo[0]["c"] >= NST - 3
                             or pair_fifo[0]["c"] <= 1))
                if po_queue and ((beat % 2 == 0 and not boundary)
                                 or not lanes_active):
                    po_group(*po_queue.pop(0))
                beat += 1
            while po_queue:
                po_group(*po_queue.pop(0))

    nc.compile()
    return nc


def _prep_inputs(q, k, v, Wq, bq, Wk, bk, Wv, bv, Wo, bo):
    import ml_dtypes
    f = np.float32
    bf = ml_dtypes.bfloat16
    xT = {}
    for g in range(DP):
        xT[("q", g)] = np.ascontiguousarray(np.asarray(q[g], f).T.astype(bf))
        xT[("k", g)] = np.ascontiguousarray(np.asarray(k[g], f).T.astype(bf))
        xT[("v", g)] = np.ascontiguousarray(np.asarray(v[g], f).T.astype(bf))
    Wq, Wk, Wv, Wo = (np.asarray(a, f) for a in (Wq, Wk, Wv, Wo))
    bq, bk, bv = (np.asarray(a, f) for a in (bq, bk, bv))
    in_maps = []
    for c in range(NCORES):
        g, r = divmod(c, TP)
        sl = slice(r * OPC, (r + 1) * OPC)
        in_maps.append({
            "xqt": xT[("q", g)], "xkt": xT[("k", g)], "xvt": xT[("v", g)],
            "wqt": np.ascontiguousarray(Wq[sl].T.astype(bf)),
            "wkt": np.ascontiguousarray(Wk[sl].T.astype(bf)),
            "wvt": np.ascontiguousarray(Wv[sl].T.astype(bf)),
            "bq": bq[sl].reshape(2, 128, 1),
            "bk": bk[sl].reshape(2, 128, 1),
            "bv": np.ascontiguousarray(np.broadcast_to(bv[sl], (128, OPC))),
            "wot": np.ascontiguousarray(Wo[:, sl].T).reshape(2, 128, D).astype(bf),
        })
    return in_maps


def kernel(q, k, v, Wq, bq, Wk, bk, Wv, bv, Wo, bo, _trace=False):
    from concourse.bass_utils import run_bass_kernel_spmd

    if "nc" not in _cache:
        _cache["nc"] = _build()
    nc = _cache["nc"]
    in_maps = _prep_inputs(q, k, v, Wq, bq, Wk, bk, Wv, bv, Wo, bo)
    res = run_bass_kernel_spmd(nc, in_maps, list(range(NCORES)), trace=_trace)
    _cache["last_exec_time_ns"] = res.exec_time_ns
    _cache["last_res"] = res
    parts = [res.results[c]["out"] for c in range(NCORES)]
    bo = np.asarray(bo, np.float32)
    out = np.empty((B, S, D), np.float32)
    for g in range(DP):
        acc = parts[g * TP].astype(np.float32)
        for r in range(1, TP):
            acc = acc + parts[g * TP + r].astype(np.float32)
        out[g] = acc + bo
    return out


# revision 27
# speedup vs baseline: 1.1809x; 1.1581x over previous
"""Multi-head attention (B=2, S=2048, D=1024, H=16) on 8 TRN2 NeuronCores.

Sharding: DP=2 over batch x TP=4 over heads (4 heads/core).
Per core: QKV projections for its 256 output dims, attention for its 4
heads on its batch, row-parallel output projection producing a partial
[2048, 1024] bf16; host sums the 4 partials per batch and adds bo.

v4 = the v1 attention core (f32r operands keep the PE dense and the HAM
clock warm; ACT runs the exp stream at ~97% busy) plus:
  - batched DMAs, all on the sync queue (1 DMA per weight tensor via a
    3D access pattern, full-row x tiles): phase A was DMA-instruction
    bound before (~70us), now ~bandwidth bound
  - phase A reordered k-proj -> q-proj(first col group) -> prefill of
    the first (hp, ic) chunk's scores+exp -> q-proj rest -> v-proj, so
    the ACT exp stream starts ~25us earlier
  - output projection interleaved per-ic into the attention loop
    (stacked/wo in bf16, PSUM->SBUF copies on DVE, bf16 output DMA):
    the old 44us serial tail mostly disappears
  - normalization copies kept off ACT (DVE reciprocal_approx_fast on a
    PE-broadcast of the denominators)
"""
import numpy as np

B, S, D = 2, 2048, 1024
HEADS, DK = 16, 64
NCORES, DP, TP = 8, 2, 4
OPC = D // TP          # 256 output dims per core
HPC = HEADS // TP      # 4 heads per core
NDC = D // 128         # 8 contraction chunks
NST = S // 128         # 16 s-tiles
NIC = S // 512         # 4 i-chunks

_cache = {}


def _build():
    import concourse.mybir as mybir
    import concourse.tile as tile
    from concourse import bacc

    F32 = mybir.dt.float32
    F32R = mybir.dt.float32r
    BF16 = mybir.dt.bfloat16
    Exp = mybir.ActivationFunctionType.Exp

    nc = bacc.Bacc("TRN2", target_bir_lowering=False, debug=False)

    xq_d = nc.dram_tensor("xqt", [D, S], BF16, kind="ExternalInput")
    xk_d = nc.dram_tensor("xkt", [D, S], BF16, kind="ExternalInput")
    xv_d = nc.dram_tensor("xvt", [D, S], BF16, kind="ExternalInput")
    wq_d = nc.dram_tensor("wqt", [D, OPC], BF16, kind="ExternalInput")
    wk_d = nc.dram_tensor("wkt", [D, OPC], BF16, kind="ExternalInput")
    wv_d = nc.dram_tensor("wvt", [D, OPC], BF16, kind="ExternalInput")
    bq_d = nc.dram_tensor("bq", [2, 128, 1], F32, kind="ExternalInput")
    bk_d = nc.dram_tensor("bk", [2, 128, 1], F32, kind="ExternalInput")
    bv_d = nc.dram_tensor("bv", [128, OPC], F32, kind="ExternalInput")
    wo_d = nc.dram_tensor("wot", [2, 128, D], BF16, kind="ExternalInput")
    out_d = nc.dram_tensor("out", [S, D], BF16, kind="ExternalOutput")

    with tile.TileContext(nc) as tc:
        from contextlib import ExitStack
        es = ExitStack()
        with es:
            wp = es.enter_context(tc.tile_pool(name="wp", bufs=1))
            acts = es.enter_context(tc.tile_pool(name="acts", bufs=1))
            xp = es.enter_context(tc.tile_pool(name="xin", bufs=1))
            ep = es.enter_context(tc.tile_pool(name="ep", bufs=1))
            sps = es.enter_context(tc.tile_pool(name="sps", bufs=2, space="PSUM"))

            # constants
            ones164 = wp.tile([1, 64], BF16, name="ones164")
            nc.vector.memset(ones164[:], 1.0)

            # persistent activations (attention core stays f32r as in v1)
            qh_st = [acts.tile([128, S], F32R, name=f"qh{h}") for h in range(2)]
            kh_st = [acts.tile([128, S], F32R, name=f"kh{h}") for h in range(2)]
            vh_all = acts.tile([128, NST * HPC * 65], F32R, name="vh_all")
            ones_cols = vh_all[:].rearrange("p (g c) -> p g c", c=65)[:, :, 64:65]
            nc.vector.memset(ones_cols.bitcast(F32), 1.0)
            stacked = [acts.tile([128, S], BF16, name=f"st{h}") for h in range(2)]

            def vh_ap(h, j):
                base = (j * HPC + h) * 65
                return vh_all[:, base:base + 65]

            def dma(dst, src):
                nc.sync.dma_start(dst, src)

            # ---------- DMAs (all up front, sync queue, batched) ----------
            def load_w(wd):
                wt = wp.tile([128, NDC * OPC], BF16, name=f"w_{wd.name}")
                dst = wt[:].rearrange("p (dcb c) -> p dcb c", c=OPC)
                srcr = wd.ap().rearrange("(dcb p) c -> p dcb c", p=128)
                dma(dst, srcr)
                return wt

            def w_ap(wt, dc, lo=0, hi=OPC):
                return wt[:, dc * OPC + lo:dc * OPC + hi]

            def load_x_rows(xd, cols=(0, S), tag="xt", w=2048):
                xt = [xp.tile([128, w], BF16, name=tag, tag=tag, bufs=8)
                      for _ in range(NDC)]
                for dc in range(NDC):
                    dma(xt[dc][:, 0:cols[1] - cols[0]],
                        xd.ap()[dc * 128:(dc + 1) * 128, cols[0]:cols[1]])
                return xt

            wk_t = load_w(wk_d)
            xk = load_x_rows(xk_d, tag="xt")
            bk_t = [wp.tile([128, 1], F32, name=f"bk{h}") for h in range(2)]
            for h in range(2):
                dma(bk_t[h][:], bk_d.ap()[h])
            wq_t = load_w(wq_d)
            xq0 = load_x_rows(xq_d, cols=(0, 512), tag="xq0", w=512)
            bq_t = [wp.tile([128, 1], F32, name=f"bq{h}") for h in range(2)]
            for h in range(2):
                dma(bq_t[h][:], bq_d.ap()[h])
            wv_t = load_w(wv_d)
            bv2 = wp.tile([128, OPC], F32, name="bv2")
            dma(bv2[:], bv_d.ap())
            xv = load_x_rows(xv_d, tag="xt")
            xqr = load_x_rows(xq_d, cols=(512, 2048), tag="xqr", w=1536)
            wo_t = [wp.tile([128, D], BF16, name=f"wo{h}") for h in range(2)]
            for h in range(2):
                dma(wo_t[h][:], wo_d.ap()[h])

            # ---------- compute helpers ----------
            def et_tile():
                return ep.tile([128, 1024], F32R, name="et", tag="et", bufs=12)

            def qk_proj_group(pool, xs, wt, bias, dest, sc):
                for hp in range(2):
                    p = pool.tile([128, 512], F32, name="pp", tag="pp")
                    for dc in range(NDC):
                        nc.tensor.matmul(
                            p[:], w_ap(wt, dc, hp * 128, (hp + 1) * 128),
                            xs(dc),
                            start=(dc == 0), stop=(dc == NDC - 1))
                    nc.vector.tensor_scalar_add(
                        dest[hp][:, sc * 512:(sc + 1) * 512], p[:],
                        bias[hp][:])

            def scores_pair(hp, ic, j):
                sp = sps.tile([128, 1024], F32, name="sp", tag="sp")
                nc.tensor.matmul(
                    sp[:, 0:512], kh_st[hp][0:64, j * 128:(j + 1) * 128],
                    qh_st[hp][0:64, ic * 512:(ic + 1) * 512],
                    start=True, stop=True, tile_position=(0, 0))
                nc.tensor.matmul(
                    sp[:, 512:1024], kh_st[hp][64:128, j * 128:(j + 1) * 128],
                    qh_st[hp][64:128, ic * 512:(ic + 1) * 512],
                    start=True, stop=True, tile_position=(64, 0))
                return sp

            def exp_tile(sp):
                et = et_tile()
                nc.scalar.activation(et[:], sp[:], Exp, scale=0.125)
                return et

            def attn_v(av, hp, j, et):
                for h2 in range(2):
                    nc.tensor.matmul(
                        av[h2][0:DK + 1, :], vh_ap(hp * 2 + h2, j),
                        et[:, h2 * 512:(h2 + 1) * 512],
                        start=(j == 0), stop=(j == NST - 1),
                        skip_group_check=True)

            # ================= phase A + prefill =================
            with tc.tile_pool(name="pps", bufs=3, space="PSUM") as pps:
                for sc in range(4):
                    qk_proj_group(
                        pps,
                        lambda dc, sc=sc: xk[dc][:, sc * 512:(sc + 1) * 512],
                        wk_t, bk_t, kh_st, sc)
                qk_proj_group(pps, lambda dc: xq0[dc][:], wq_t, bq_t,
                              qh_st, 0)

                # prefill: scores+exp for the first half of (ic=0, hp=0)
                # while q1/v-proj run (SBUF bounds the f32r et buffer count)
                prefill_ets = {j: exp_tile(scores_pair(0, 0, j))
                               for j in range(8)}

                for sc in range(1, 4):
                    qk_proj_group(
                        pps,
                        lambda dc, sc=sc: xqr[dc][:, (sc - 1) * 512:sc * 512],
                        wq_t, bq_t, qh_st, sc)

                for st in range(NST):
                    pv = pps.tile([128, OPC], F32, name="pv", tag="pp")
                    for dc in range(NDC):
                        nc.tensor.matmul(
                            pv[:], xv[dc][:, st * 128:(st + 1) * 128],
                            w_ap(wv_t, dc), start=(dc == 0),
                            stop=(dc == NDC - 1))
                    dst = vh_all[:, st * HPC * 65:(st + 1) * HPC * 65]
                    dst = dst.rearrange("p (h c) -> p h c", h=HPC)[:, :, 0:64]
                    nc.vector.tensor_add(
                        dst, pv[:].rearrange("p (h c) -> p h c", h=HPC),
                        bv2[:].rearrange("p (h c) -> p h c", h=HPC))

            # ================= phase B with interleaved out-proj ==========
            with tc.tile_pool(name="avps", bufs=3, space="PSUM") as avps, \
                 tc.tile_pool(name="mbp", bufs=1, space="PSUM") as mbp:

                def norm(av, hp, ic):
                    den = [ep.tile([1, 512], BF16, name="den", tag="den",
                                   bufs=4) for _ in range(2)]
                    for h2 in range(2):
                        nc.vector.tensor_copy(den[h2][:],
                                              av[h2][DK:DK + 1, :])
                    r2 = mbp.tile([128, 512], F32, name="r2", tag="mb")
                    nc.tensor.matmul(r2[0:64, :], ones164[:], den[0][:],
                                     start=True, stop=True,
                                     tile_position=(0, 0))
                    nc.tensor.matmul(r2[64:128, :], ones164[:], den[1][:],
                                     start=True, stop=True,
                                     tile_position=(0, 64))
                    r2s = ep.tile([128, 512], F32, name="r2s", tag="r2s",
                                  bufs=2)
                    nc.vector.reciprocal_approx_fast(r2s[:], r2[:])
                    for h2 in range(2):
                        nc.vector.tensor_mul(
                            stacked[hp][h2 * 64:(h2 + 1) * 64,
                                        ic * 512:(ic + 1) * 512],
                            av[h2][0:DK, :], r2s[h2 * 64:(h2 + 1) * 64, :])

                def po_group(it, mc):
                    po = mbp.tile([128, 512], F32, name="po", tag="mb")
                    for hp in range(2):
                        nc.tensor.matmul(
                            po[:], stacked[hp][:, it * 128:(it + 1) * 128],
                            wo_t[hp][:, mc * 512:(mc + 1) * 512],
                            start=(hp == 0), stop=(hp == 1))
                    ot = ep.tile([128, 512], BF16, name="ot", tag="ot",
                                 bufs=4)
                    nc.vector.tensor_copy(ot[:], po[:])
                    dma(out_d.ap()[it * 128:(it + 1) * 128,
                                   mc * 512:(mc + 1) * 512], ot[:])

                po_queue = []
                for ic in range(NIC):
                    for hp in range(2):
                        av = [avps.tile([DK + 1, 512], F32, name="av",
                                        tag="av") for _ in range(2)]
                        if ic == 0 and hp == 0:
                            ets = prefill_ets
                            for j in range(NST):
                                attn_v(av, hp, j, ets.pop(j))
                                if j + 8 < NST:
                                    ets[j + 8] = exp_tile(
                                        scores_pair(hp, ic, j + 8))
                                if po_queue and j % 2 == 0:
                                    po_group(*po_queue.pop(0))
                        else:
                            ets = {}
                            for j in range(2):
                                ets[j] = exp_tile(scores_pair(hp, ic, j))
                            for j in range(NST):
                                attn_v(av, hp, j, ets.pop(j))
                                if j + 2 < NST:
                                    ets[j + 2] = exp_tile(
                                        scores_pair(hp, ic, j + 2))
                                if po_queue and j % 2 == 0:
                                    po_group(*po_queue.pop(0))
                        norm(av, hp, ic)
                    po_queue.extend(
                        (it, mc) for it in range(ic * 4, ic * 4 + 4)
                        for mc in range(2))
                while po_queue:
                    po_group(*po_queue.pop(0))

    nc.compile()
    return nc


def _prep_inputs(q, k, v, Wq, bq, Wk, bk, Wv, bv, Wo, bo):
    import ml_dtypes
    f = np.float32
    bf = ml_dtypes.bfloat16
    xT = {}
    for g in range(DP):
        xT[("q", g)] = np.ascontiguousarray(np.asarray(q[g], f).T.astype(bf))
        xT[("k", g)] = np.ascontiguousarray(np.asarray(k[g], f).T.astype(bf))
        xT[("v", g)] = np.ascontiguousarray(np.asarray(v[g], f).T.astype(bf))
    Wq, Wk, Wv, Wo = (np.asarray(a, f) for a in (Wq, Wk, Wv, Wo))
    bq, bk, bv = (np.asarray(a, f) for a in (bq, bk, bv))
    in_maps = []
    for c in range(NCORES):
        g, r = divmod(c, TP)
        sl = slice(r * OPC, (r + 1) * OPC)
        in_maps.append({
            "xqt": xT[("q", g)], "xkt": xT[("k", g)], "xvt": xT[("v", g)],
            "wqt": np.ascontiguousarray(Wq[sl].T.astype(bf)),
            "wkt": np.ascontiguousarray(Wk[sl].T.astype(bf)),
            "wvt": np.ascontiguousarray(Wv[sl].T.astype(bf)),
            "bq": bq[sl].reshape(2, 128, 1),
            "bk": bk[sl].reshape(2, 128, 1),
            "bv": np.ascontiguousarray(np.broadcast_to(bv[sl], (128, OPC))),
            "wot": np.ascontiguousarray(Wo[:, sl].T).reshape(2, 128, D).astype(bf),
        })
    return in_maps


def kernel(q, k, v, Wq, bq, Wk, bk, Wv, bv, Wo, bo, _trace=False):
    from concourse.bass_utils import run_bass_kernel_spmd

    if "nc" not in _cache:
        _cache["nc"] = _build()
    nc = _cache["nc"]
    in_maps = _prep_inputs(q, k, v, Wq, bq, Wk, bk, Wv, bv, Wo, bo)
    res = run_bass_kernel_spmd(nc, in_maps, list(range(NCORES)), trace=_trace)
    _cache["last_exec_time_ns"] = res.exec_time_ns
    _cache["last_res"] = res
    parts = [res.results[c]["out"] for c in range(NCORES)]
    bo = np.asarray(bo, np.float32)
    out = np.empty((B, S, D), np.float32)
    for g in range(DP):
        acc = parts[g * TP].astype(np.float32)
        for r in range(1, TP):
            acc = acc + parts[g * TP + r].astype(np.float32)
        out[g] = acc + bo
    return out
